# revision 17
# baseline (speedup 1.0000x reference)
"""DinoMask2Former NMS-detection kernel for 8 Trainium2 NeuronCores.

Strategy (contraction-dim sharding):
  - pred_masks [300, 320, 320] f32 is sharded by image ROWS (h) across the 8
    cores: core i gets h in [40*i, 40*(i+1)) -> a [300, 12800] f32 slice.
  - Each core: cast-loads its slice to bf16, binarizes (sign-exact in bf16),
    emits the binary mask shard, and computes a partial [300, 300] mask-
    intersection GEMM (binary masks in bf16, fp32 PSUM accumulation -> exact
    integer counts). A ones-column fused into the moving operand yields the
    areas; 40 h-indicator weight columns fused into the third weight tile
    yield per-h pixel counts (y-extents). A max-accumulated [300, 320]
    vector gives the x-extents, and one scalar-engine tanh pass with
    accumulation gives the quality-rescore sum
        sum_hw sigmoid(x)*[x>0] = 0.5*sum tanh(relu(x)/2) + 0.5*area.
  - The [HW, Q] operand the TensorEngine needs is built with the DMA x-bar
    transpose (bf16, SBUF->SBUF) so no compute engine pays for transposes.
  - Host: sums the tiny per-core partials (the "all-gather" of the QxQ IoU
    matrix), computes softmax scores/labels from the tiny pred_logits, then
    runs the cheap, inherently sequential greedy NMS and assembles the
    outputs exactly as the reference does.
"""

import os
import sys

import numpy as np

for _p in ("/opt/trn_rl_repo",):
    if _p not in sys.path:
        sys.path.append(_p)

N_CORES = 8
Q = 300
H = 320
W = 320
NHC = H // N_CORES          # 40 h-rows per core
HWC = NHC * W               # 12800 elements per query per core
# variable-size groups (h-rows): small first groups shorten the pipeline
# ramp, small last groups shorten the matmul tail
GROUPS = [4, 8, 8, 8, 8, 4]
NG = len(GROUPS)
NCHT = HWC // 128           # 100 transposed chunks per core
XTW = 304                   # xt cols: 300 q | ones | 3 pad
QT = [(0, 128, 128), (128, 128, 128), (256, 44, 48)]  # (q0, qn, padded)

MASK_THRESHOLD = 0.5
# smallest f32 x with sigmoid_f32(x) > 0.5 under the XLA-CPU logistic the
# reference uses (bit-bisected); plain x > 0 differs for a few tiny positives
SIG_X0 = np.float32(8.9406974e-08)
SCORE_THRESHOLD = 0.05
TOPK_PER_IMAGE = 100
IOU_THRESHOLD = 0.85

_NC_CACHE = None


def _aux_host():
    """Pre-transposed h-indicator weight columns, per global chunk:
    auxT[p, cg*40 + j] = 1 iff hw-row cg*128+p lies in local h-row j."""
    import ml_dtypes
    aux = np.zeros((128, NCHT * NHC), np.float32)
    for cg in range(NCHT):
        for p in range(128):
            j = (cg * 128 + p) // W
            aux[p, cg * NHC + j] = 1.0
    return aux.astype(ml_dtypes.bfloat16)


def _patch_ldw_opt():
    # bass_utils hardcodes --enable-ldw-opt=false; with it off every
    # LDWEIGHTS->MATMUL pair serializes (next LDW waits for the running MM).
    # All our weight tiles are full 128-column loads, which the LDW
    # optimizer supports. Gated by env for A/B testing.
    from concourse import bass_utils as _bu
    if getattr(_bu, "_ldw_patched", False):
        return
    _orig = _bu.run_command

    def _patched(cmd, *a, **kw):
        cmd = [c.replace("--enable-ldw-opt=false", "--enable-ldw-opt=true")
               if isinstance(c, str) else c for c in cmd]
        return _orig(cmd, *a, **kw)

    _bu.run_command = _patched
    _bu._ldw_patched = True


def _build():
    from contextlib import ExitStack

    import concourse.tile as tile
    from concourse import bacc, mybir

    if os.environ.get("BASS_LDW_OPT", "0") == "1":
        _patch_ldw_opt()

    nc = bacc.Bacc("TRN2", target_bir_lowering=False, debug=False,
                   num_devices=N_CORES)

    xm = nc.dram_tensor("xm", [Q, HWC], mybir.dt.float32, kind="ExternalInput")
    aux_d = nc.dram_tensor("aux", [128, NCHT * NHC], mybir.dt.bfloat16,
                           kind="ExternalInput")
    binm = nc.dram_tensor("binm", [Q, HWC], mybir.dt.bfloat16, kind="ExternalOutput")
    interA = nc.dram_tensor("interA", [128, 301], mybir.dt.float32, kind="ExternalOutput")
    interB = nc.dram_tensor("interB", [128, 173], mybir.dt.float32, kind="ExternalOutput")
    interC = nc.dram_tensor("interC", [44, 301], mybir.dt.float32, kind="ExternalOutput")
    hsumt = nc.dram_tensor("hsumt", [NHC, 300], mybir.dt.float32, kind="ExternalOutput")
    qacc = nc.dram_tensor("qacc", [Q, NG], mybir.dt.float32, kind="ExternalOutput")
    xany = nc.dram_tensor("xany", [Q, W], mybir.dt.bfloat16, kind="ExternalOutput")

    f32 = mybir.dt.float32
    bf16 = mybir.dt.bfloat16
    Alu = mybir.AluOpType
    Act = mybir.ActivationFunctionType

    with tile.TileContext(nc) as tc, ExitStack() as ctx:
        io = ctx.enter_context(tc.tile_pool(name="io", bufs=3))
        xtp = ctx.enter_context(tc.tile_pool(name="xtp", bufs=3))
        pers = ctx.enter_context(tc.tile_pool(name="pers", bufs=1))
        psum = ctx.enter_context(tc.tile_pool(name="psum", bufs=1, space="PSUM"))

        # persistent tiles
        qa_t = [pers.tile([128, NG], f32, tag=f"qa{t}", name=f"qa{t}")
                for t in range(3)]
        acc_t = [pers.tile([128, 2560], bf16, tag=f"acc{t}", name=f"acc{t}")
                 for t in range(3)]
        for t in range(3):
            nc.vector.memset(acc_t[t][:, :], 0.0)
        aux_s = pers.tile([128, NCHT * NHC], bf16, tag="aux", name="aux_s")
        nc.scalar.dma_start(aux_s[:, :], aux_d.ap()[:, :])

        # PSUM accumulators (one bank each, held across all 100 chunks):
        #   psA: inter rows 0:128   x cols 0:300 + area col
        #   psB: inter rows 128:256 x cols 128:300 + area col
        #   psC: rows 0:44 inter rows 256:300 (full); row 44 colsums;
        #        rows 45:85 per-h counts (y-extents)
        psA = psum.tile([128, 301], f32, tag="psA", name="psA")
        psB = psum.tile([128, 173], f32, tag="psB", name="psB")
        psC = psum.tile([44, 301], f32, tag="psC", name="psC")
        psH = psum.tile([NHC, 301], f32, tag="psH", name="psH")

        cbase = 0
        for g, GH in enumerate(GROUPS):
            hwg = GH * W
            nch = hwg // 128
            c0 = sum(GROUPS[:g]) * W
            bins = []
            for t, (q0, qn, pn) in enumerate(QT):
                # cast-load f32 -> bf16 (SWDGE; gpsimd stream is loads only)
                x = io.tile([pn, 2560], bf16, tag=f"x{t}", name=f"x{t}_{g}",
                            bufs=4)
                nc.gpsimd.dma_start(x[:qn, 0:hwg],
                                    xm.ap()[q0:q0 + qn, c0:c0 + hwg])

                # binarize (sigmoid(x) > 0.5  <=>  x >= SIG_X0)
                b = io.tile([pn, 2560], bf16, tag=f"b{t}", name=f"b{t}_{g}", bufs=2)
                if pn > qn:
                    nc.gpsimd.memset(b[32:pn, 0:hwg], 0.0)
                nc.vector.tensor_scalar(b[:qn, 0:hwg], x[:qn, 0:hwg],
                                        float(SIG_X0), None, Alu.is_ge)
                bins.append(b)

                # relu(x) then tanh(relu/2) in place, accumulating the
                # quality partial
                r = io.tile([pn, 2560], bf16, tag=f"r{t}", name=f"r{t}_{g}",
                            bufs=2)
                nc.vector.tensor_scalar(r[:qn, 0:hwg], x[:qn, 0:hwg], 0.0,
                                        None, Alu.max)
                nc.scalar.activation(r[:qn, 0:hwg], r[:qn, 0:hwg], Act.Tanh,
                                     scale=0.5,
                                     accum_out=qa_t[t][:qn, g:g + 1])

                # x-extent accumulator: max over h rows (w-phase mod 320)
                nc.vector.tensor_tensor(acc_t[t][:qn, 0:hwg],
                                        acc_t[t][:qn, 0:hwg],
                                        b[:qn, 0:hwg], Alu.max)

                # binary-mask shard output (bf16; host reads nonzero bytes).
                # ACT HWDGE ring so the gpsimd stream stays pure loads.
                nc.scalar.dma_start(binm.ap()[q0:q0 + qn, c0:c0 + hwg],
                                    b[:qn, 0:hwg])

            # DMA x-bar transpose: [q, hw] -> [128, nch, XTW] (bf16)
            xt = xtp.tile([128, 20, XTW], bf16, tag="xt", name=f"xt_{g}")
            for t, (q0, qn, pn) in enumerate(QT):
                nc.sync.dma_start(xt[:, 0:nch, q0:q0 + pn],
                                  bins[t][0:pn, 0:hwg], transpose=True)
            # fused area column + zeroed pad
            nc.vector.memset(xt[:, 0:nch, 300:301], 1.0)
            nc.vector.memset(xt[:, 0:nch, 301:304], 0.0)

            # partial intersection GEMM (+ per-h counts from auxT)
            for c in range(nch):
                cg = cbase + c
                first = (cg == 0)
                last = (cg == NCHT - 1)
                nc.tensor.matmul(psA[:, 0:301], xt[:, c, 0:128],
                                 xt[:, c, 0:301], start=first, stop=last)
                nc.tensor.matmul(psB[:, 0:173], xt[:, c, 128:256],
                                 xt[:, c, 128:301], start=first, stop=last)
                nc.tensor.matmul(psC[:44, 0:301], xt[:, c, 256:300],
                                 xt[:, c, 0:301], start=first, stop=last)
                nc.tensor.matmul(psH[:, 0:301],
                                 aux_s[:, cg * NHC:(cg + 1) * NHC],
                                 xt[:, c, 0:301], start=first, stop=last)
            cbase += nch

        # epilogue: fold x-extent accumulator 2560 -> 320 and ship stats
        for t, (q0, qn, pn) in enumerate(QT):
            a = acc_t[t]
            nc.vector.tensor_tensor(a[:qn, 0:1280], a[:qn, 0:1280],
                                    a[:qn, 1280:2560], Alu.max)
            nc.vector.tensor_tensor(a[:qn, 0:640], a[:qn, 0:640],
                                    a[:qn, 640:1280], Alu.max)
            nc.vector.tensor_tensor(a[:qn, 0:320], a[:qn, 0:320],
                                    a[:qn, 320:640], Alu.max)
            nc.scalar.dma_start(xany.ap()[q0:q0 + qn, :], a[:qn, 0:320])
            nc.scalar.dma_start(qacc.ap()[q0:q0 + qn, :], qa_t[t][:qn, :])

        # inter partials: PSUM -> SBUF -> DRAM
        oA = io.tile([128, 301], f32, tag="oA", name="oA")
        nc.vector.tensor_copy(oA[:, :], psA[:, 0:301])
        nc.scalar.dma_start(interA.ap()[:, :], oA[:, :])
        oB = io.tile([128, 173], f32, tag="oB", name="oB")
        nc.vector.tensor_copy(oB[:, :], psB[:, 0:173])
        nc.scalar.dma_start(interB.ap()[:, :], oB[:, :])
        oC = io.tile([44, 301], f32, tag="oC", name="oC")
        nc.vector.tensor_copy(oC[:44, :], psC[:44, 0:301])
        nc.scalar.dma_start(interC.ap()[:, :], oC[0:44, :])
        oH = io.tile([NHC, 300], f32, tag="oH", name="oH")
        nc.vector.tensor_copy(oH[:NHC, :], psH[:NHC, 0:300])
        nc.scalar.dma_start(hsumt.ap()[:, :], oH[:, :])

    nc.compile()
    return nc


def _get_nc():
    global _NC_CACHE
    if _NC_CACHE is None:
        _NC_CACHE = _build()
    return _NC_CACHE


def _run_on_cores(masks_f32):
    from concourse.bass_utils import run_bass_kernel_spmd

    nc = _get_nc()
    aux = _aux_host()
    in_maps = []
    for i in range(N_CORES):
        shard = np.ascontiguousarray(
            masks_f32[:, i * NHC:(i + 1) * NHC, :]).reshape(Q, HWC)
        in_maps.append({"xm": shard, "aux": aux})
    res = run_bass_kernel_spmd(nc, in_maps, list(range(N_CORES)))
    return res


def kernel(pred_logits, pred_masks):
    logits = np.asarray(pred_logits, dtype=np.float32)
    masks = np.asarray(pred_masks, dtype=np.float32)
    assert masks.shape == (Q, H, W)

    res = _run_on_cores(masks)
    outs = res.results

    # ---- gather / unshard ----
    bin_masks = np.concatenate(
        [outs[i]["binm"].view(np.uint16).reshape(Q, NHC, W) != 0
         for i in range(N_CORES)], axis=1)

    A = np.zeros((128, 301), np.float32)
    B = np.zeros((128, 173), np.float32)
    C = np.zeros((44, 301), np.float32)
    hsum_full = np.zeros((Q, H), np.float32)
    qual_sum = np.zeros((Q,), np.float32)
    xany = np.zeros((Q, W), np.float32)
    for i in range(N_CORES):
        A += outs[i]["interA"]
        B += outs[i]["interB"]
        C += outs[i]["interC"]
        hsum_full[:, i * NHC:(i + 1) * NHC] = outs[i]["hsumt"].T
        qual_sum += outs[i]["qacc"].sum(axis=1, dtype=np.float32)
        np.maximum(xany, outs[i]["xany"].astype(np.float32), out=xany)

    inter = np.zeros((Q, Q), np.float32)
    inter[0:128, 0:300] = A[:, 0:300]
    inter[128:256, 128:300] = B[:, 0:172]
    inter[256:300, 0:300] = C[:, 0:300]
    inter[128:256, 0:128] = A[:, 128:256].T

    area = np.concatenate([A[:, 300], B[:, 172], C[:, 300]]).astype(np.float32)
    y_any = hsum_full > 0
    x_any = xany > 0
    qual_num = np.float32(0.5) * qual_sum + np.float32(0.5) * area
    quality = qual_num / np.maximum(area, np.float32(1.0))

    # Boundary audit: the device thresholds bf16(x) >= SIG_X0; for the
    # (astronomically rare) pixels whose bf16 rounding crosses SIG_X0 the
    # f32 comparison can disagree. Patch those pixels and the affected
    # per-query stats.
    lo = SIG_X0 * np.float32(0.995)
    hi = SIG_X0 * np.float32(1.005)
    cand = (masks > lo) & (masks < hi)
    if cand.any():
        correct = masks[cand] >= SIG_X0
        if np.any(correct != bin_masks[cand]):
            cq, ch, cw = np.where(cand)
            flipped_q = set()
            for q, h, w in zip(cq, ch, cw):
                c = masks[q, h, w] >= SIG_X0
                if bin_masks[q, h, w] != c:
                    bin_masks[q, h, w] = c
                    flipped_q.add(int(q))
            if flipped_q:
                flat_f = bin_masks.reshape(Q, -1).astype(np.float32)
                for q in flipped_q:
                    row = flat_f @ flat_f[q]
                    inter[q, :] = row
                    inter[:, q] = row
                    area[q] = flat_f[q].sum(dtype=np.float32)
                    x_any[q] = bin_masks[q].any(axis=0)
                    y_any[q] = bin_masks[q].any(axis=1)

    # ---- class scores (tiny) ----
    m = logits.max(axis=1, keepdims=True)
    e = np.exp(logits - m, dtype=np.float32)
    z = e.sum(axis=1, dtype=np.float32)
    probs80 = e[:, :80] / z[:, None]
    scores = probs80.max(axis=1)
    labels = probs80.argmax(axis=1).astype(np.int32)

    # ---- greedy NMS in descending-score order (cheap, sequential) ----
    order = np.argsort(-scores, kind="stable")
    inter_s = inter[np.ix_(order, order)]
    area_s = area[order]
    union = np.maximum(area_s[:, None] + area_s[None, :] - inter_s,
                       np.float32(1.0))
    iou = inter_s / union
    valid_s = (scores[order] >= np.float32(SCORE_THRESHOLD)) & (area_s > 0)

    keep_s = np.zeros(Q, bool)
    count = 0
    sup = iou > IOU_THRESHOLD
    for i in range(Q):
        overlaps = bool(np.any(keep_s & sup[i]))
        k = bool(valid_s[i]) and (not overlaps) and (count < TOPK_PER_IMAGE)
        keep_s[i] = k
        count += int(k)
    keep = np.zeros(Q, bool)
    keep[order] = keep_s

    final_scores = np.where(keep, scores * quality,
                            np.float32(0.0)).astype(np.float32)

    # ---- boxes from extents ----
    xs = np.arange(W, dtype=np.float32)
    ys = np.arange(H, dtype=np.float32)
    x0 = np.where(x_any, xs[None, :], np.float32(W)).min(axis=1)
    x1 = np.where(x_any, xs[None, :], np.float32(-1.0)).max(axis=1) + 1.0
    y0 = np.where(y_any, ys[None, :], np.float32(H)).min(axis=1)
    y1 = np.where(y_any, ys[None, :], np.float32(-1.0)).max(axis=1) + 1.0
    boxes = np.stack([x0, y0, x1, y1], axis=-1).astype(np.float32)
    boxes = np.where((keep & (area > 0))[:, None], boxes, np.float32(0.0))

    return keep, final_scores, labels, bin_masks, boxes


# revision 18
# speedup vs baseline: 1.0220x; 1.0220x over previous
"""DinoMask2Former NMS-detection kernel for 8 Trainium2 NeuronCores.

Strategy (contraction-dim sharding):
  - pred_masks [300, 320, 320] f32 is sharded by image ROWS (h) across the 8
    cores: core i gets h in [40*i, 40*(i+1)) -> a [300, 12800] f32 slice.
  - Each core: cast-loads its slice to bf16, binarizes (sign-exact in bf16),
    emits the binary mask shard, and computes a partial [300, 300] mask-
    intersection GEMM (binary masks in bf16, fp32 PSUM accumulation -> exact
    integer counts). A ones-column fused into the moving operand yields the
    areas; 40 h-indicator weight columns fused into the third weight tile
    yield per-h pixel counts (y-extents). A max-accumulated [300, 320]
    vector gives the x-extents, and one scalar-engine tanh pass with
    accumulation gives the quality-rescore sum
        sum_hw sigmoid(x)*[x>0] = 0.5*sum tanh(relu(x)/2) + 0.5*area.
  - The [HW, Q] operand the TensorEngine needs is built with the DMA x-bar
    transpose (bf16, SBUF->SBUF) so no compute engine pays for transposes.
  - Host: sums the tiny per-core partials (the "all-gather" of the QxQ IoU
    matrix), computes softmax scores/labels from the tiny pred_logits, then
    runs the cheap, inherently sequential greedy NMS and assembles the
    outputs exactly as the reference does.
"""

import os
import sys

import numpy as np

for _p in ("/opt/trn_rl_repo",):
    if _p not in sys.path:
        sys.path.append(_p)

N_CORES = 8
Q = 300
H = 320
W = 320
NHC = H // N_CORES          # 40 h-rows per core
HWC = NHC * W               # 12800 elements per query per core
# variable-size groups (h-rows): small first groups shorten the pipeline
# ramp, small last groups shorten the matmul tail
GROUPS = [4, 8, 8, 8, 8, 4]
NG = len(GROUPS)
NCHT = HWC // 128           # 100 transposed chunks per core
XTW = 304                   # xt cols: 300 q | ones | 3 pad
QT = [(0, 128, 128), (128, 128, 128), (256, 44, 48)]  # (q0, qn, padded)

MASK_THRESHOLD = 0.5
# smallest f32 x with sigmoid_f32(x) > 0.5 under the XLA-CPU logistic the
# reference uses (bit-bisected); plain x > 0 differs for a few tiny positives
SIG_X0 = np.float32(8.9406974e-08)
SCORE_THRESHOLD = 0.05
TOPK_PER_IMAGE = 100
IOU_THRESHOLD = 0.85

_NC_CACHE = None


def _aux_host():
    """Pre-transposed h-indicator weight columns, per global chunk:
    auxT[p, cg*40 + j] = 1 iff hw-row cg*128+p lies in local h-row j."""
    import ml_dtypes
    aux = np.zeros((128, NCHT * NHC), np.float32)
    for cg in range(NCHT):
        for p in range(128):
            j = (cg * 128 + p) // W
            aux[p, cg * NHC + j] = 1.0
    return aux.astype(ml_dtypes.bfloat16)


def _patch_ldw_opt():
    # bass_utils hardcodes --enable-ldw-opt=false; with it off every
    # LDWEIGHTS->MATMUL pair serializes (next LDW waits for the running MM).
    # All our weight tiles are full 128-column loads, which the LDW
    # optimizer supports. Gated by env for A/B testing.
    from concourse import bass_utils as _bu
    if getattr(_bu, "_ldw_patched", False):
        return
    _orig = _bu.run_command

    def _patched(cmd, *a, **kw):
        cmd = [c.replace("--enable-ldw-opt=false", "--enable-ldw-opt=true")
               if isinstance(c, str) else c for c in cmd]
        return _orig(cmd, *a, **kw)

    _bu.run_command = _patched
    _bu._ldw_patched = True


def _build():
    from contextlib import ExitStack

    import concourse.tile as tile
    from concourse import bacc, mybir

    if os.environ.get("BASS_LDW_OPT", "0") == "1":
        _patch_ldw_opt()

    nc = bacc.Bacc("TRN2", target_bir_lowering=False, debug=False,
                   num_devices=N_CORES)

    xm = nc.dram_tensor("xm", [Q, HWC], mybir.dt.float32, kind="ExternalInput")
    aux_d = nc.dram_tensor("aux", [128, NCHT * NHC], mybir.dt.bfloat16,
                           kind="ExternalInput")
    binm = nc.dram_tensor("binm", [Q, HWC], mybir.dt.bfloat16, kind="ExternalOutput")
    interA = nc.dram_tensor("interA", [128, 301], mybir.dt.float32, kind="ExternalOutput")
    interB = nc.dram_tensor("interB", [128, 173], mybir.dt.float32, kind="ExternalOutput")
    interC = nc.dram_tensor("interC", [44, 301], mybir.dt.float32, kind="ExternalOutput")
    hsumt = nc.dram_tensor("hsumt", [NHC, 300], mybir.dt.float32, kind="ExternalOutput")
    qacc = nc.dram_tensor("qacc", [Q, NG], mybir.dt.float32, kind="ExternalOutput")
    xany = nc.dram_tensor("xany", [Q, W], mybir.dt.bfloat16, kind="ExternalOutput")

    f32 = mybir.dt.float32
    bf16 = mybir.dt.bfloat16
    Alu = mybir.AluOpType
    Act = mybir.ActivationFunctionType

    with tile.TileContext(nc) as tc, ExitStack() as ctx:
        io = ctx.enter_context(tc.tile_pool(name="io", bufs=3))
        xtp = ctx.enter_context(tc.tile_pool(name="xtp", bufs=2))
        pers = ctx.enter_context(tc.tile_pool(name="pers", bufs=1))
        psum = ctx.enter_context(tc.tile_pool(name="psum", bufs=1, space="PSUM"))

        # persistent tiles
        qa_t = [pers.tile([128, NG], f32, tag=f"qa{t}", name=f"qa{t}")
                for t in range(3)]
        acc_t = [pers.tile([128, 2560], bf16, tag=f"acc{t}", name=f"acc{t}")
                 for t in range(3)]
        for t in range(3):
            nc.vector.memset(acc_t[t][:, :], 0.0)
        aux_s = pers.tile([128, NCHT * NHC], bf16, tag="aux", name="aux_s")
        nc.gpsimd.dma_start(aux_s[:, :], aux_d.ap()[:, :])

        # PSUM accumulators (one bank each, held across all 100 chunks):
        #   psA: inter rows 0:128   x cols 0:300 + area col
        #   psB: inter rows 128:256 x cols 128:300 + area col
        #   psC: rows 0:44 inter rows 256:300 (full); row 44 colsums;
        #        rows 45:85 per-h counts (y-extents)
        psA = psum.tile([128, 301], f32, tag="psA", name="psA")
        psB = psum.tile([128, 173], f32, tag="psB", name="psB")
        psC = psum.tile([44, 301], f32, tag="psC", name="psC")
        psH = psum.tile([NHC, 301], f32, tag="psH", name="psH")

        cbase = 0
        for g, GH in enumerate(GROUPS):
            hwg = GH * W
            nch = hwg // 128
            c0 = sum(GROUPS[:g]) * W
            bins = []
            for t, (q0, qn, pn) in enumerate(QT):
                # cast-load f32 -> bf16 (SWDGE; gpsimd stream is loads only)
                x = io.tile([pn, 2560], bf16, tag=f"x{t}", name=f"x{t}_{g}",
                            bufs=4)
                nc.gpsimd.dma_start(x[:qn, 0:hwg],
                                    xm.ap()[q0:q0 + qn, c0:c0 + hwg])

                # binarize (sigmoid(x) > 0.5  <=>  x >= SIG_X0)
                b = io.tile([pn, 2560], bf16, tag=f"b{t}", name=f"b{t}_{g}")
                if pn > qn:
                    nc.gpsimd.memset(b[32:pn, 0:hwg], 0.0)
                nc.vector.tensor_scalar(b[:qn, 0:hwg], x[:qn, 0:hwg],
                                        float(SIG_X0), None, Alu.is_ge)
                bins.append(b)

                # relu(x) then tanh(relu/2) in place, accumulating the
                # quality partial
                r = io.tile([pn, 2560], bf16, tag=f"r{t}", name=f"r{t}_{g}",
                            bufs=2)
                nc.vector.tensor_scalar(r[:qn, 0:hwg], x[:qn, 0:hwg], 0.0,
                                        None, Alu.max)
                nc.scalar.activation(r[:qn, 0:hwg], r[:qn, 0:hwg], Act.Tanh,
                                     scale=0.5,
                                     accum_out=qa_t[t][:qn, g:g + 1])

                # x-extent accumulator: max over h rows (w-phase mod 320)
                nc.vector.tensor_tensor(acc_t[t][:qn, 0:hwg],
                                        acc_t[t][:qn, 0:hwg],
                                        b[:qn, 0:hwg], Alu.max)

                # binary-mask shard output (bf16; host reads nonzero bytes).
                # ACT HWDGE ring so the gpsimd stream stays pure loads.
                nc.scalar.dma_start(binm.ap()[q0:q0 + qn, c0:c0 + hwg],
                                    b[:qn, 0:hwg])

            # DMA x-bar transpose: [q, hw] -> [128, nch, XTW] (bf16)
            xt = xtp.tile([128, 20, XTW], bf16, tag="xt", name=f"xt_{g}")
            for t, (q0, qn, pn) in enumerate(QT):
                nc.sync.dma_start(xt[:, 0:nch, q0:q0 + pn],
                                  bins[t][0:pn, 0:hwg], transpose=True)
            # fused area column + zeroed pad
            nc.vector.memset(xt[:, 0:nch, 300:301], 1.0)
            nc.vector.memset(xt[:, 0:nch, 301:304], 0.0)

            # partial intersection GEMM (+ per-h counts from auxT)
            for c in range(nch):
                cg = cbase + c
                first = (cg == 0)
                last = (cg == NCHT - 1)
                nc.tensor.matmul(psA[:, 0:301], xt[:, c, 0:128],
                                 xt[:, c, 0:301], start=first, stop=last)
                nc.tensor.matmul(psB[:, 0:173], xt[:, c, 128:256],
                                 xt[:, c, 128:301], start=first, stop=last)
                nc.tensor.matmul(psC[:44, 0:301], xt[:, c, 256:300],
                                 xt[:, c, 0:301], start=first, stop=last)
                nc.tensor.matmul(psH[:, 0:301],
                                 aux_s[:, cg * NHC:(cg + 1) * NHC],
                                 xt[:, c, 0:301], start=first, stop=last)
            cbase += nch

        # epilogue: fold x-extent accumulator 2560 -> 320 and ship stats
        for t, (q0, qn, pn) in enumerate(QT):
            a = acc_t[t]
            nc.vector.tensor_tensor(a[:qn, 0:1280], a[:qn, 0:1280],
                                    a[:qn, 1280:2560], Alu.max)
            nc.vector.tensor_tensor(a[:qn, 0:640], a[:qn, 0:640],
                                    a[:qn, 640:1280], Alu.max)
            nc.vector.tensor_tensor(a[:qn, 0:320], a[:qn, 0:320],
                                    a[:qn, 320:640], Alu.max)
            nc.scalar.dma_start(xany.ap()[q0:q0 + qn, :], a[:qn, 0:320])
            nc.scalar.dma_start(qacc.ap()[q0:q0 + qn, :], qa_t[t][:qn, :])

        # inter partials: PSUM -> SBUF -> DRAM
        oA = io.tile([128, 301], f32, tag="oA", name="oA")
        nc.vector.tensor_copy(oA[:, :], psA[:, 0:301])
        nc.scalar.dma_start(interA.ap()[:, :], oA[:, :])
        oB = io.tile([128, 173], f32, tag="oB", name="oB")
        nc.vector.tensor_copy(oB[:, :], psB[:, 0:173])
        nc.scalar.dma_start(interB.ap()[:, :], oB[:, :])
        oC = io.tile([44, 301], f32, tag="oC", name="oC")
        nc.vector.tensor_copy(oC[:44, :], psC[:44, 0:301])
        nc.scalar.dma_start(interC.ap()[:, :], oC[0:44, :])
        oH = io.tile([NHC, 300], f32, tag="oH", name="oH")
        nc.vector.tensor_copy(oH[:NHC, :], psH[:NHC, 0:300])
        nc.scalar.dma_start(hsumt.ap()[:, :], oH[:, :])

    nc.compile()
    return nc


def _get_nc():
    global _NC_CACHE
    if _NC_CACHE is None:
        _NC_CACHE = _build()
    return _NC_CACHE


def _run_on_cores(masks_f32):
    from concourse.bass_utils import run_bass_kernel_spmd

    nc = _get_nc()
    aux = _aux_host()
    in_maps = []
    for i in range(N_CORES):
        shard = np.ascontiguousarray(
            masks_f32[:, i * NHC:(i + 1) * NHC, :]).reshape(Q, HWC)
        in_maps.append({"xm": shard, "aux": aux})
    res = run_bass_kernel_spmd(nc, in_maps, list(range(N_CORES)))
    return res


def kernel(pred_logits, pred_masks):
    logits = np.asarray(pred_logits, dtype=np.float32)
    masks = np.asarray(pred_masks, dtype=np.float32)
    assert masks.shape == (Q, H, W)

    res = _run_on_cores(masks)
    outs = res.results

    # ---- gather / unshard ----
    bin_masks = np.concatenate(
        [outs[i]["binm"].view(np.uint16).reshape(Q, NHC, W) != 0
         for i in range(N_CORES)], axis=1)

    A = np.zeros((128, 301), np.float32)
    B = np.zeros((128, 173), np.float32)
    C = np.zeros((44, 301), np.float32)
    hsum_full = np.zeros((Q, H), np.float32)
    qual_sum = np.zeros((Q,), np.float32)
    xany = np.zeros((Q, W), np.float32)
    for i in range(N_CORES):
        A += outs[i]["interA"]
        B += outs[i]["interB"]
        C += outs[i]["interC"]
        hsum_full[:, i * NHC:(i + 1) * NHC] = outs[i]["hsumt"].T
        qual_sum += outs[i]["qacc"].sum(axis=1, dtype=np.float32)
        np.maximum(xany, outs[i]["xany"].astype(np.float32), out=xany)

    inter = np.zeros((Q, Q), np.float32)
    inter[0:128, 0:300] = A[:, 0:300]
    inter[128:256, 128:300] = B[:, 0:172]
    inter[256:300, 0:300] = C[:, 0:300]
    inter[128:256, 0:128] = A[:, 128:256].T

    area = np.concatenate([A[:, 300], B[:, 172], C[:, 300]]).astype(np.float32)
    y_any = hsum_full > 0
    x_any = xany > 0
    qual_num = np.float32(0.5) * qual_sum + np.float32(0.5) * area
    quality = qual_num / np.maximum(area, np.float32(1.0))

    # Boundary audit: the device thresholds bf16(x) >= SIG_X0; for the
    # (astronomically rare) pixels whose bf16 rounding crosses SIG_X0 the
    # f32 comparison can disagree. Patch those pixels and the affected
    # per-query stats.
    lo = SIG_X0 * np.float32(0.995)
    hi = SIG_X0 * np.float32(1.005)
    cand = (masks > lo) & (masks < hi)
    if cand.any():
        correct = masks[cand] >= SIG_X0
        if np.any(correct != bin_masks[cand]):
            cq, ch, cw = np.where(cand)
            flipped_q = set()
            for q, h, w in zip(cq, ch, cw):
                c = masks[q, h, w] >= SIG_X0
                if bin_masks[q, h, w] != c:
                    bin_masks[q, h, w] = c
                    flipped_q.add(int(q))
            if flipped_q:
                flat_f = bin_masks.reshape(Q, -1).astype(np.float32)
                for q in flipped_q:
                    row = flat_f @ flat_f[q]
                    inter[q, :] = row
                    inter[:, q] = row
                    area[q] = flat_f[q].sum(dtype=np.float32)
                    x_any[q] = bin_masks[q].any(axis=0)
                    y_any[q] = bin_masks[q].any(axis=1)

    # ---- class scores (tiny) ----
    m = logits.max(axis=1, keepdims=True)
    e = np.exp(logits - m, dtype=np.float32)
    z = e.sum(axis=1, dtype=np.float32)
    probs80 = e[:, :80] / z[:, None]
    scores = probs80.max(axis=1)
    labels = probs80.argmax(axis=1).astype(np.int32)

    # ---- greedy NMS in descending-score order (cheap, sequential) ----
    order = np.argsort(-scores, kind="stable")
    inter_s = inter[np.ix_(order, order)]
    area_s = area[order]
    union = np.maximum(area_s[:, None] + area_s[None, :] - inter_s,
                       np.float32(1.0))
    iou = inter_s / union
    valid_s = (scores[order] >= np.float32(SCORE_THRESHOLD)) & (area_s > 0)

    keep_s = np.zeros(Q, bool)
    count = 0
    sup = iou > IOU_THRESHOLD
    for i in range(Q):
        overlaps = bool(np.any(keep_s & sup[i]))
        k = bool(valid_s[i]) and (not overlaps) and (count < TOPK_PER_IMAGE)
        keep_s[i] = k
        count += int(k)
    keep = np.zeros(Q, bool)
    keep[order] = keep_s

    final_scores = np.where(keep, scores * quality,
                            np.float32(0.0)).astype(np.float32)

    # ---- boxes from extents ----
    xs = np.arange(W, dtype=np.float32)
    ys = np.arange(H, dtype=np.float32)
    x0 = np.where(x_any, xs[None, :], np.float32(W)).min(axis=1)
    x1 = np.where(x_any, xs[None, :], np.float32(-1.0)).max(axis=1) + 1.0
    y0 = np.where(y_any, ys[None, :], np.float32(H)).min(axis=1)
    y1 = np.where(y_any, ys[None, :], np.float32(-1.0)).max(axis=1) + 1.0
    boxes = np.stack([x0, y0, x1, y1], axis=-1).astype(np.float32)
    boxes = np.where((keep & (area > 0))[:, None], boxes, np.float32(0.0))

    return keep, final_scores, labels, bin_masks, boxes


# revision 19
# speedup vs baseline: 1.1147x; 1.0906x over previous
"""DinoMask2Former NMS-detection kernel for 8 Trainium2 NeuronCores.

Strategy (contraction-dim sharding):
  - pred_masks [300, 320, 320] f32 is sharded by image ROWS (h) across the 8
    cores: core i gets h in [40*i, 40*(i+1)) -> a [300, 12800] f32 slice.
  - Each core: cast-loads its slice to bf16, binarizes (sign-exact in bf16),
    emits the binary mask shard, and computes a partial [300, 300] mask-
    intersection GEMM (binary masks in bf16, fp32 PSUM accumulation -> exact
    integer counts). A ones-column fused into the moving operand yields the
    areas; 40 h-indicator weight columns fused into the third weight tile
    yield per-h pixel counts (y-extents). A max-accumulated [300, 320]
    vector gives the x-extents, and one scalar-engine tanh pass with
    accumulation gives the quality-rescore sum
        sum_hw sigmoid(x)*[x>0] = 0.5*sum tanh(relu(x)/2) + 0.5*area.
  - The [HW, Q] operand the TensorEngine needs is built with the DMA x-bar
    transpose (bf16, SBUF->SBUF) so no compute engine pays for transposes.
  - Host: sums the tiny per-core partials (the "all-gather" of the QxQ IoU
    matrix), computes softmax scores/labels from the tiny pred_logits, then
    runs the cheap, inherently sequential greedy NMS and assembles the
    outputs exactly as the reference does.
"""

import os
import sys

import numpy as np

for _p in ("/opt/trn_rl_repo",):
    if _p not in sys.path:
        sys.path.append(_p)

N_CORES = 8
Q = 300
H = 320
W = 320
NHC = H // N_CORES          # 40 h-rows per core
HWC = NHC * W               # 12800 elements per query per core
# variable-size groups (h-rows): small first groups shorten the pipeline
# ramp, small last groups shorten the matmul tail
GROUPS = [4, 8, 8, 8, 8, 4]
NG = len(GROUPS)
NCHT = HWC // 128           # 100 transposed chunks per core
XTW = 304                   # xt cols: 300 q | ones | 3 pad
QT = [(0, 128, 128), (128, 128, 128), (256, 44, 48)]  # (q0, qn, padded)

MASK_THRESHOLD = 0.5
# smallest f32 x with sigmoid_f32(x) > 0.5 under the XLA-CPU logistic the
# reference uses (bit-bisected); plain x > 0 differs for a few tiny positives
SIG_X0 = np.float32(8.9406974e-08)
SCORE_THRESHOLD = 0.05
TOPK_PER_IMAGE = 100
IOU_THRESHOLD = 0.85

_NC_CACHE = None


def _aux_host():
    """Pre-transposed h-indicator weight columns, per global chunk:
    auxT[p, cg*40 + j] = 1 iff hw-row cg*128+p lies in local h-row j."""
    import ml_dtypes
    aux = np.zeros((128, NCHT * NHC), np.float32)
    for cg in range(NCHT):
        for p in range(128):
            j = (cg * 128 + p) // W
            aux[p, cg * NHC + j] = 1.0
    return aux.astype(ml_dtypes.bfloat16)


def _patch_ldw_opt():
    # bass_utils hardcodes --enable-ldw-opt=false; with it off every
    # LDWEIGHTS->MATMUL pair serializes (next LDW waits for the running MM).
    # All our weight tiles are full 128-column loads, which the LDW
    # optimizer supports. Gated by env for A/B testing.
    from concourse import bass_utils as _bu
    if getattr(_bu, "_ldw_patched", False):
        return
    _orig = _bu.run_command

    def _patched(cmd, *a, **kw):
        cmd = [c.replace("--enable-ldw-opt=false", "--enable-ldw-opt=true")
               if isinstance(c, str) else c for c in cmd]
        return _orig(cmd, *a, **kw)

    _bu.run_command = _patched
    _bu._ldw_patched = True


def _build():
    from contextlib import ExitStack

    import concourse.tile as tile
    from concourse import bacc, mybir

    if os.environ.get("BASS_LDW_OPT", "0") == "1":
        _patch_ldw_opt()

    nc = bacc.Bacc("TRN2", target_bir_lowering=False, debug=False,
                   num_devices=N_CORES)

    xm = nc.dram_tensor("xm", [Q, HWC], mybir.dt.float32, kind="ExternalInput")
    aux_d = nc.dram_tensor("aux", [128, NCHT * NHC], mybir.dt.bfloat16,
                           kind="ExternalInput")
    binm = nc.dram_tensor("binm", [Q, HWC], mybir.dt.bfloat16, kind="ExternalOutput")
    interA = nc.dram_tensor("interA", [128, 301], mybir.dt.float32, kind="ExternalOutput")
    interB = nc.dram_tensor("interB", [128, 173], mybir.dt.float32, kind="ExternalOutput")
    interC = nc.dram_tensor("interC", [44, 301], mybir.dt.float32, kind="ExternalOutput")
    hsumt = nc.dram_tensor("hsumt", [NHC, 300], mybir.dt.float32, kind="ExternalOutput")
    qacc = nc.dram_tensor("qacc", [Q, NG], mybir.dt.float32, kind="ExternalOutput")
    xany = nc.dram_tensor("xany", [Q, W], mybir.dt.bfloat16, kind="ExternalOutput")

    f32 = mybir.dt.float32
    bf16 = mybir.dt.bfloat16
    Alu = mybir.AluOpType
    Act = mybir.ActivationFunctionType

    with tile.TileContext(nc) as tc, ExitStack() as ctx:
        io = ctx.enter_context(tc.tile_pool(name="io", bufs=3))
        xtp = ctx.enter_context(tc.tile_pool(name="xtp", bufs=2))
        pers = ctx.enter_context(tc.tile_pool(name="pers", bufs=1))
        psum = ctx.enter_context(tc.tile_pool(name="psum", bufs=1, space="PSUM"))

        # persistent tiles
        qa_t = [pers.tile([128, NG], f32, tag=f"qa{t}", name=f"qa{t}")
                for t in range(3)]
        acc_t = [pers.tile([128, 2560], bf16, tag=f"acc{t}", name=f"acc{t}")
                 for t in range(3)]
        for t in range(3):
            nc.vector.memset(acc_t[t][:, :], 0.0)
        aux_s = pers.tile([128, NCHT * NHC], bf16, tag="aux", name="aux_s")
        nc.gpsimd.dma_start(aux_s[:, :], aux_d.ap()[:, :])

        # PSUM accumulators (one bank each, held across all 100 chunks):
        #   psA: inter rows 0:128   x cols 0:300 + area col
        #   psB: inter rows 128:256 x cols 128:300 + area col
        #   psC: rows 0:44 inter rows 256:300 (full); row 44 colsums;
        #        rows 45:85 per-h counts (y-extents)
        psA = psum.tile([128, 301], f32, tag="psA", name="psA")
        psB = psum.tile([128, 173], f32, tag="psB", name="psB")
        psC = psum.tile([44, 301], f32, tag="psC", name="psC")
        psH = psum.tile([NHC, 301], f32, tag="psH", name="psH")

        cbase = 0
        for g, GH in enumerate(GROUPS):
            hwg = GH * W
            nch = hwg // 128
            c0 = sum(GROUPS[:g]) * W
            bins = []
            xs = []
            for t, (q0, qn, pn) in enumerate(QT):
                # cast-load f32 -> bf16 (SWDGE; gpsimd stream is loads only)
                x = io.tile([pn, 2560], bf16, tag=f"x{t}", name=f"x{t}_{g}",
                            bufs=4)
                nc.gpsimd.dma_start(x[:qn, 0:hwg],
                                    xm.ap()[q0:q0 + qn, c0:c0 + hwg])
                xs.append(x)

            # binarize first (sigmoid(x) > 0.5  <=>  x >= SIG_X0) so the
            # transposes' DVE-counter waits cover only the bins
            for t, (q0, qn, pn) in enumerate(QT):
                b = io.tile([pn, 2560], bf16, tag=f"b{t}", name=f"b{t}_{g}")
                if pn > qn:
                    nc.gpsimd.memset(b[32:pn, 0:hwg], 0.0)
                nc.vector.tensor_scalar(b[:qn, 0:hwg], xs[t][:qn, 0:hwg],
                                        float(SIG_X0), None, Alu.is_ge)
                bins.append(b)

            # DMA x-bar transpose: [q, hw] -> [128, nch, XTW] (bf16)
            xt = xtp.tile([128, 20, XTW], bf16, tag="xt", name=f"xt_{g}")
            for t, (q0, qn, pn) in enumerate(QT):
                nc.sync.dma_start(xt[:, 0:nch, q0:q0 + pn],
                                  bins[t][0:pn, 0:hwg], transpose=True)
            # fused area column + zeroed pad (before the rest of the DVE
            # work so the matmuls' DVE-counter waits clear early)
            nc.vector.memset(xt[:, 0:nch, 300:301], 1.0)
            nc.vector.memset(xt[:, 0:nch, 301:304], 0.0)

            for t, (q0, qn, pn) in enumerate(QT):
                # relu(x) then tanh(relu/2) in place -> quality partial
                r = io.tile([pn, 2560], bf16, tag=f"r{t}", name=f"r{t}_{g}",
                            bufs=2)
                nc.vector.tensor_scalar(r[:qn, 0:hwg], xs[t][:qn, 0:hwg],
                                        0.0, None, Alu.max)
                nc.scalar.activation(r[:qn, 0:hwg], r[:qn, 0:hwg], Act.Tanh,
                                     scale=0.5,
                                     accum_out=qa_t[t][:qn, g:g + 1])
                # x-extent accumulator: max over h rows (w-phase mod 320)
                nc.vector.tensor_tensor(acc_t[t][:qn, 0:hwg],
                                        acc_t[t][:qn, 0:hwg],
                                        bins[t][:qn, 0:hwg], Alu.max)
                # binary-mask shard output (bf16; host reads nonzero bytes)
                nc.scalar.dma_start(binm.ap()[q0:q0 + qn, c0:c0 + hwg],
                                    bins[t][:qn, 0:hwg])

            # partial intersection GEMM (+ per-h counts from auxT)
            for c in range(nch):
                cg = cbase + c
                first = (cg == 0)
                last = (cg == NCHT - 1)
                nc.tensor.matmul(psA[:, 0:301], xt[:, c, 0:128],
                                 xt[:, c, 0:301], start=first, stop=last)
                nc.tensor.matmul(psB[:, 0:173], xt[:, c, 128:256],
                                 xt[:, c, 128:301], start=first, stop=last)
                nc.tensor.matmul(psC[:44, 0:301], xt[:, c, 256:300],
                                 xt[:, c, 0:301], start=first, stop=last)
                nc.tensor.matmul(psH[:, 0:301],
                                 aux_s[:, cg * NHC:(cg + 1) * NHC],
                                 xt[:, c, 0:301], start=first, stop=last)
            cbase += nch

        # epilogue: fold x-extent accumulator 2560 -> 320 and ship stats
        for t, (q0, qn, pn) in enumerate(QT):
            a = acc_t[t]
            nc.vector.tensor_tensor(a[:qn, 0:1280], a[:qn, 0:1280],
                                    a[:qn, 1280:2560], Alu.max)
            nc.vector.tensor_tensor(a[:qn, 0:640], a[:qn, 0:640],
                                    a[:qn, 640:1280], Alu.max)
            nc.vector.tensor_tensor(a[:qn, 0:320], a[:qn, 0:320],
                                    a[:qn, 320:640], Alu.max)
            nc.scalar.dma_start(xany.ap()[q0:q0 + qn, :], a[:qn, 0:320])
            nc.scalar.dma_start(qacc.ap()[q0:q0 + qn, :], qa_t[t][:qn, :])

        # inter partials: PSUM -> SBUF -> DRAM
        oA = io.tile([128, 301], f32, tag="oA", name="oA")
        nc.vector.tensor_copy(oA[:, :], psA[:, 0:301])
        nc.scalar.dma_start(interA.ap()[:, :], oA[:, :])
        oB = io.tile([128, 173], f32, tag="oB", name="oB")
        nc.vector.tensor_copy(oB[:, :], psB[:, 0:173])
        nc.scalar.dma_start(interB.ap()[:, :], oB[:, :])
        oC = io.tile([44, 301], f32, tag="oC", name="oC")
        nc.vector.tensor_copy(oC[:44, :], psC[:44, 0:301])
        nc.scalar.dma_start(interC.ap()[:, :], oC[0:44, :])
        oH = io.tile([NHC, 300], f32, tag="oH", name="oH")
        nc.vector.tensor_copy(oH[:NHC, :], psH[:NHC, 0:300])
        nc.scalar.dma_start(hsumt.ap()[:, :], oH[:, :])

    nc.compile()
    return nc


def _get_nc():
    global _NC_CACHE
    if _NC_CACHE is None:
        _NC_CACHE = _build()
    return _NC_CACHE


def _run_on_cores(masks_f32):
    from concourse.bass_utils import run_bass_kernel_spmd

    nc = _get_nc()
    aux = _aux_host()
    in_maps = []
    for i in range(N_CORES):
        shard = np.ascontiguousarray(
            masks_f32[:, i * NHC:(i + 1) * NHC, :]).reshape(Q, HWC)
        in_maps.append({"xm": shard, "aux": aux})
    res = run_bass_kernel_spmd(nc, in_maps, list(range(N_CORES)))
    return res


def kernel(pred_logits, pred_masks):
    logits = np.asarray(pred_logits, dtype=np.float32)
    masks = np.asarray(pred_masks, dtype=np.float32)
    assert masks.shape == (Q, H, W)

    res = _run_on_cores(masks)
    outs = res.results

    # ---- gather / unshard ----
    bin_masks = np.concatenate(
        [outs[i]["binm"].view(np.uint16).reshape(Q, NHC, W) != 0
         for i in range(N_CORES)], axis=1)

    A = np.zeros((128, 301), np.float32)
    B = np.zeros((128, 173), np.float32)
    C = np.zeros((44, 301), np.float32)
    hsum_full = np.zeros((Q, H), np.float32)
    qual_sum = np.zeros((Q,), np.float32)
    xany = np.zeros((Q, W), np.float32)
    for i in range(N_CORES):
        A += outs[i]["interA"]
        B += outs[i]["interB"]
        C += outs[i]["interC"]
        hsum_full[:, i * NHC:(i + 1) * NHC] = outs[i]["hsumt"].T
        qual_sum += outs[i]["qacc"].sum(axis=1, dtype=np.float32)
        np.maximum(xany, outs[i]["xany"].astype(np.float32), out=xany)

    inter = np.zeros((Q, Q), np.float32)
    inter[0:128, 0:300] = A[:, 0:300]
    inter[128:256, 128:300] = B[:, 0:172]
    inter[256:300, 0:300] = C[:, 0:300]
    inter[128:256, 0:128] = A[:, 128:256].T

    area = np.concatenate([A[:, 300], B[:, 172], C[:, 300]]).astype(np.float32)
    y_any = hsum_full > 0
    x_any = xany > 0
    qual_num = np.float32(0.5) * qual_sum + np.float32(0.5) * area
    quality = qual_num / np.maximum(area, np.float32(1.0))

    # Boundary audit: the device thresholds bf16(x) >= SIG_X0; for the
    # (astronomically rare) pixels whose bf16 rounding crosses SIG_X0 the
    # f32 comparison can disagree. Patch those pixels and the affected
    # per-query stats.
    lo = SIG_X0 * np.float32(0.995)
    hi = SIG_X0 * np.float32(1.005)
    cand = (masks > lo) & (masks < hi)
    if cand.any():
        correct = masks[cand] >= SIG_X0
        if np.any(correct != bin_masks[cand]):
            cq, ch, cw = np.where(cand)
            flipped_q = set()
            for q, h, w in zip(cq, ch, cw):
                c = masks[q, h, w] >= SIG_X0
                if bin_masks[q, h, w] != c:
                    bin_masks[q, h, w] = c
                    flipped_q.add(int(q))
            if flipped_q:
                flat_f = bin_masks.reshape(Q, -1).astype(np.float32)
                for q in flipped_q:
                    row = flat_f @ flat_f[q]
                    inter[q, :] = row
                    inter[:, q] = row
                    area[q] = flat_f[q].sum(dtype=np.float32)
                    x_any[q] = bin_masks[q].any(axis=0)
                    y_any[q] = bin_masks[q].any(axis=1)

    # ---- class scores (tiny) ----
    m = logits.max(axis=1, keepdims=True)
    e = np.exp(logits - m, dtype=np.float32)
    z = e.sum(axis=1, dtype=np.float32)
    probs80 = e[:, :80] / z[:, None]
    scores = probs80.max(axis=1)
    labels = probs80.argmax(axis=1).astype(np.int32)

    # ---- greedy NMS in descending-score order (cheap, sequential) ----
    order = np.argsort(-scores, kind="stable")
    inter_s = inter[np.ix_(order, order)]
    area_s = area[order]
    union = np.maximum(area_s[:, None] + area_s[None, :] - inter_s,
                       np.float32(1.0))
    iou = inter_s / union
    valid_s = (scores[order] >= np.float32(SCORE_THRESHOLD)) & (area_s > 0)

    keep_s = np.zeros(Q, bool)
    count = 0
    sup = iou > IOU_THRESHOLD
    for i in range(Q):
        overlaps = bool(np.any(keep_s & sup[i]))
        k = bool(valid_s[i]) and (not overlaps) and (count < TOPK_PER_IMAGE)
        keep_s[i] = k
        count += int(k)
    keep = np.zeros(Q, bool)
    keep[order] = keep_s

    final_scores = np.where(keep, scores * quality,
                            np.float32(0.0)).astype(np.float32)

    # ---- boxes from extents ----
    xs = np.arange(W, dtype=np.float32)
    ys = np.arange(H, dtype=np.float32)
    x0 = np.where(x_any, xs[None, :], np.float32(W)).min(axis=1)
    x1 = np.where(x_any, xs[None, :], np.float32(-1.0)).max(axis=1) + 1.0
    y0 = np.where(y_any, ys[None, :], np.float32(H)).min(axis=1)
    y1 = np.where(y_any, ys[None, :], np.float32(-1.0)).max(axis=1) + 1.0
    boxes = np.stack([x0, y0, x1, y1], axis=-1).astype(np.float32)
    boxes = np.where((keep & (area > 0))[:, None], boxes, np.float32(0.0))

    return keep, final_scores, labels, bin_masks, boxes
